# revision 7
# baseline (speedup 1.0000x reference)
"""Trainium2 Bass kernel for nn_Net_64982855188859 (ECC graph-conv net).

Network (per graph g of B=8, N=128 nodes, F=16 feats, S=8 edge feats, C=32):
  mask = x[..., -1:], h = x[..., :16]
  h = leaky_relu(ECC1(h, a, e) * mask)          ECC: per-edge MLP -> filters
  h = relu(ECC2(h, a, e)) * mask
  out = softmax(relu(mean_nodes(h) @ dw) @ ow)

Key algebraic restructuring (exact, not approximate):
 1. The einsum out[i,c] = sum_j a[i,j] * sum_k h2[i,j,k] * xW[j,k,c] where
    xW[j,k,c] = sum_f x[j,f] * w1c[k, f*C+c]  (w1c factored through x, which
    turns the huge [N,N,F*C] per-edge filter tensor into a [N, 32*C] tensor).
 2. a[i,j] >= 0 is a per-edge scalar, so it commutes through the per-edge
    ReLU MLP rows:  a*relu(relu(e@w1a)@w1b) = relu(relu((a*e)@w1a)@w1b).
    The host pre-multiplies e by a (eA), removing all masking work on device.
    This requires the MLP biases to be zero, which they structurally are in
    this problem (asserted on host).
 3. Per-core work = one graph (data-parallel over B=8 across 8 cores).

Device layout (edge order j-major: edge = j*128 + i):
  eA_sb  [128, 4096]: partitions 32r+s (s<8) hold edge-feature s of row-tile r;
         free = (round g, 512 edges); 4-way row-tiled K=8 matmuls -> h1.
  h1     [128, 512] psum banks: (half, k') x (j4, i); relu -> sbuf.
  h2     quad-pair psum [128, 256]: partitions (j4, k), free (quad, i);
         relu -> h2A_sb [128, 4096] = 32 contraction chunks [(j4,k), i].
  xW/yW  [128 (j), 1024 (k,c)] via matmul, restructured to [(j4,k), (t,c)]
         chunks through a DRAM round-trip (pure-AP strided DMA).
  contraction: out1[c, i] += xWr[:, 32t:32t+32].T @ h2A[:, 128t:128t+128]
         accumulated over 32 chunks in one PSUM bank, + bias outer product
         + root term, all in one accumulation group.
"""

import numpy as np

import concourse.bass as bass
import concourse.bacc as bacc
import concourse.mybir as mybir
import concourse.tile as tile
from concourse.bass_utils import run_bass_kernel_spmd

F32 = mybir.dt.float32
AF = mybir.ActivationFunctionType
ALU = mybir.AluOpType

B, N, F, S, C = 8, 128, 16, 8, 32
E = N * N               # 16384 edges per graph
NROUND = 8              # h1 rounds; each processes 2048 edges (16 j's)

# wpack column layout ([128, WCOLS] fp32, shared across cores)
W1A = slice(0, 64)        # w1a stacked at rows 32r..32r+8
W1B = slice(64, 96)       # w1b at rows 0:64 and 64:128
W2A = slice(96, 160)
W2B = slice(160, 192)
ROOT1 = slice(192, 224)   # rows 0:16
ROOT2 = slice(224, 256)   # rows 0:32
DW = slice(256, 320)      # rows 0:32
OW = slice(320, 330)      # rows 0:64
DB = slice(330, 331)      # rows 0:64  (column vector)
BIAS1 = slice(336, 368)   # row 0 (row vector)
BIAS2 = slice(368, 400)   # row 0
OB = slice(400, 410)      # row 0
WCOLS = 416


def _ecc_mlp_layer(nc, tc, pools, wp_sb, eA_sb, wa_cols, wb_cols, h2A_sb, evict_ct):
    """Emit the per-edge MLP for one ECC layer: eA -> h1 -> h2A_sb chunks."""
    pool_h1p, pool_h1s, pool_qp = pools
    for g in range(NROUND):
        # --- h1: 4 row-tiled K=8 matmuls into 2 psum banks ---
        banks = [pool_h1p.tile([128, 512], F32, tag="h1p", name=f"h1p{g}a"),
                 pool_h1p.tile([128, 512], F32, tag="h1p", name=f"h1p{g}b")]
        for r in range(4):
            nc.tensor.matmul(
                out=banks[r // 2][64 * (r % 2):64 * (r % 2) + 64, :],
                lhsT=wp_sb[32 * r:32 * r + 8, wa_cols],
                rhs=eA_sb[32 * r:32 * r + 8, 512 * g:512 * (g + 1)],
                start=True, stop=True,
                tile_position=(32 * r, 64 * (r % 2)),
            )
        # --- relu evict to sbuf (alternate ACT/DVE) ---
        h1s = []
        for bi, bank in enumerate(banks):
            t = pool_h1s.tile([128, 512], F32, tag="h1s", name=f"h1s{g}_{bi}")
            if (evict_ct[0] + bi) % 2 == 0:
                nc.scalar.activation(out=t[:, :], in_=bank[:, :], func=AF.Relu)
            else:
                nc.vector.tensor_scalar_max(t[:, :], bank[:, :], 0.0)
            h1s.append(t)
        evict_ct[0] += 1

        # --- h2: 16 matmuls K=64 into 2 quad-pair banks ---
        # qp[0] holds quads 4g+0 (from banks[0] half0) and 4g+2 (banks[1] half0)
        # qp[1] holds quads 4g+1 (banks[0] half1) and 4g+3 (banks[1] half1)
        qp = [pool_qp.tile([128, 256], F32, tag="qp", name=f"qp{g}a"),
              pool_qp.tile([128, 256], F32, tag="qp", name=f"qp{g}b")]
        for j4 in range(4):
            for r in range(4):
                half = r % 2            # rows 64*half .. +64
                bank_i = r // 2
                nc.tensor.matmul(
                    out=qp[half][32 * j4:32 * j4 + 32,
                                 128 * bank_i:128 * bank_i + 128],
                    lhsT=wp_sb[64 * half:64 * half + 64, wb_cols],
                    rhs=h1s[bank_i][64 * half:64 * half + 64,
                                    128 * j4:128 * j4 + 128],
                    start=True, stop=True,
                    tile_position=(64 * half, 32 * j4),
                )
        # --- relu evict quad pairs into h2A_sb chunks ---
        # qp[half] free 0:128 -> chunk t=4g+half, free 128:256 -> t=4g+2+half
        for half in range(2):
            base = h2A_sb[:, 512 * g + 128 * half:]
            out_ap = bass.AP(
                tensor=base.tensor, offset=base.offset,
                ap=[base.ap[0], [256, 2], [1, 128]],
            )
            if (evict_ct[0] + half) % 2 == 0:
                nc.scalar.activation(
                    out=out_ap, in_=qp[half][:, :].rearrange("p (q i) -> p q i", q=2),
                    func=AF.Relu)
            else:
                nc.vector.tensor_scalar_max(
                    out_ap, qp[half][:, :].rearrange("p (q i) -> p q i", q=2), 0.0)
        evict_ct[0] += 1


def _xw_restructure(nc, tc, pool_big, pool_xw, dram_pool, lhsT, rhs, name):
    """out [128 (j), 1024 (k,c)] = lhsT.T @ rhs, restructured to
    [(j4 k), (t c)] chunk layout via a DRAM round-trip. Returns sbuf tile."""
    ps = pool_big.tile([128, 1024], F32, tag="big")
    for h in range(2):
        nc.tensor.matmul(out=ps[:, 512 * h:512 * (h + 1)], lhsT=lhsT,
                         rhs=rhs[:, 512 * h:512 * (h + 1)], start=True, stop=True)
    flat = pool_xw.tile([128, 1024], F32, tag=f"{name}_flat")
    nc.scalar.activation(out=flat[:, 0:512], in_=ps[:, 0:512], func=AF.Copy)
    nc.vector.tensor_copy(flat[:, 512:1024], ps[:, 512:1024])
    dscratch = dram_pool.tile([128, 1024], F32, tag=f"{name}_dram")
    nc.sync.dma_start(out=dscratch[:, :], in_=flat[:, :])
    restr = pool_xw.tile([128, 1024], F32, tag=f"{name}_restr")
    dsr = dscratch[:, :].rearrange("(t j4) (k c) -> j4 k t c", j4=4, c=32)
    for j4 in range(4):
        nc.sync.dma_start(
            out=restr[32 * j4:32 * j4 + 32, :].rearrange(
                "k (t c) -> k t c", c=32),
            in_=dsr[j4],
        )
    return restr


def _contraction(nc, out_psum, bias_row, ones_sb, xwr_sb, h2A_sb, root_lhsT, root_rhs):
    """out1[c, i] = bias ⊗ ones + sum_t xwr_t.T @ h2A_t + root.T @ root_rhs."""
    nc.tensor.matmul(out=out_psum[:, :], lhsT=bias_row, rhs=ones_sb,
                     start=True, stop=False, skip_group_check=True)
    for t in range(32):
        nc.tensor.matmul(
            out=out_psum[:, :],
            lhsT=xwr_sb[:, 32 * t:32 * t + 32],
            rhs=h2A_sb[:, 128 * t:128 * t + 128],
            start=False, stop=False, skip_group_check=True,
        )
    nc.tensor.matmul(out=out_psum[:, :], lhsT=root_lhsT, rhs=root_rhs,
                     start=False, stop=True, skip_group_check=True)


def build_nc(loop_n: int | None = None):
    nc = bacc.Bacc("TRN2", target_bir_lowering=False, debug=False)
    eA_d = nc.dram_tensor("eA", [S, E], F32, kind="ExternalInput").ap()
    xc_d = nc.dram_tensor("xc", [17, 128], F32, kind="ExternalInput").ap()
    wp_d = nc.dram_tensor("wpack", [128, WCOLS], F32, kind="ExternalInput").ap()
    w1p_d = nc.dram_tensor("w1p", [16, 1024], F32, kind="ExternalInput").ap()
    w2p_d = nc.dram_tensor("w2p", [32, 1024], F32, kind="ExternalInput").ap()
    out_d = nc.dram_tensor("out", [1, 10], F32, kind="ExternalOutput").ap()

    with tile.TileContext(nc) as tc:
        def body():
            with (
                tc.tile_pool(name="consts", bufs=1) as consts,
                tc.tile_pool(name="eA", bufs=1) as pool_eA,
                tc.tile_pool(name="h1p", bufs=2, space="PSUM") as pool_h1p,
                tc.tile_pool(name="h1s", bufs=4) as pool_h1s,
                tc.tile_pool(name="qp", bufs=3, space="PSUM") as pool_qp,
                tc.tile_pool(name="big", bufs=1, space="PSUM") as pool_big,
                tc.tile_pool(name="outp", bufs=1, space="PSUM") as pool_out,
                tc.tile_pool(name="xw", bufs=1) as pool_xw,
                tc.tile_pool(name="h2A", bufs=1) as pool_h2A,
                tc.tile_pool(name="misc", bufs=1) as pool_misc,
                tc.tile_pool(name="dram", bufs=1, space="DRAM") as dram_pool,
            ):
                # ---- load constants ----
                wp_sb = consts.tile([128, WCOLS], F32)
                nc.sync.dma_start(out=wp_sb[:, :], in_=wp_d)
                w1p_sb = consts.tile([16, 1024], F32)
                nc.sync.dma_start(out=w1p_sb[:, :], in_=w1p_d)
                w2p_sb = consts.tile([32, 1024], F32)
                nc.sync.dma_start(out=w2p_sb[:, :], in_=w2p_d)
                xc_sb = consts.tile([17, 128], F32)
                nc.sync.dma_start(out=xc_sb[:, :], in_=xc_d)
                xT = xc_sb[0:16, :]
                mask_rep = consts.tile([32, 128], F32)
                nc.sync.dma_start(
                    out=mask_rep[:, :],
                    in_=bass.AP(tensor=xc_d.tensor, offset=16 * 128,
                                ap=[[0, 32], [1, 128]]),
                )
                ones_sb = consts.tile([1, 128], F32)
                nc.vector.memset(ones_sb[:, :], 1.0)

                # ---- load eA (row-stacked for 4-way row tiling) ----
                eA_sb = pool_eA.tile([128, 4096], F32)
                eA_r = eA_d.rearrange("s (g r c) -> r s g c", g=8, r=4, c=512)
                for r in range(4):
                    nc.sync.dma_start(
                        out=eA_sb[32 * r:32 * r + 8, :].rearrange(
                            "s (g c) -> s g c", g=8),
                        in_=eA_r[r],
                    )

                # ---- xW = x @ W1p, restructured ----
                xwr_sb = _xw_restructure(nc, tc, pool_big, pool_xw, dram_pool,
                                         xT, w1p_sb[:, :], "xw")

                # ---- layer-1 MLP over edges ----
                h2A_sb = pool_h2A.tile([128, 4096], F32, tag="h2A_l1")
                evict_ct = [0]
                _ecc_mlp_layer(nc, tc, (pool_h1p, pool_h1s, pool_qp), wp_sb,
                               eA_sb, W1A, W1B, h2A_sb, evict_ct)

                # ---- layer-1 contraction + finish ----
                out1 = pool_out.tile([32, 128], F32, tag="outp")
                _contraction(nc, out1, wp_sb[0:1, BIAS1], ones_sb[:, :],
                             xwr_sb, h2A_sb, wp_sb[0:16, ROOT1], xT)
                z_sb = pool_misc.tile([32, 128], F32, tag="z")
                nc.vector.tensor_mul(z_sb[:, :], out1[:, :], mask_rep[:, :])
                y1_sb = pool_misc.tile([32, 128], F32, tag="y1")
                nc.vector.scalar_tensor_tensor(
                    out=y1_sb[:, :], in0=z_sb[:, :], scalar=0.05, in1=z_sb[:, :],
                    op0=ALU.mult, op1=ALU.max)

                # ---- yW = y1 @ W2p, restructured ----
                ywr_sb = _xw_restructure(nc, tc, pool_big, pool_xw, dram_pool,
                                         y1_sb[:, :], w2p_sb[:, :], "yw")

                # ---- layer-2 MLP over edges ----
                g2A_sb = pool_h2A.tile([128, 4096], F32, tag="h2A_l2")
                _ecc_mlp_layer(nc, tc, (pool_h1p, pool_h1s, pool_qp), wp_sb,
                               eA_sb, W2A, W2B, g2A_sb, evict_ct)

                # ---- layer-2 contraction + finish ----
                out2 = pool_out.tile([32, 128], F32, tag="outp")
                _contraction(nc, out2, wp_sb[0:1, BIAS2], ones_sb[:, :],
                             ywr_sb, g2A_sb, wp_sb[0:32, ROOT2], y1_sb[:, :])
                r2_sb = pool_misc.tile([32, 128], F32, tag="r2")
                nc.scalar.activation(out=r2_sb[:, :], in_=out2[:, :], func=AF.Relu)
                h2f_sb = pool_misc.tile([32, 128], F32, tag="h2f")
                gv_sb = pool_misc.tile([32, 1], F32, tag="gv")
                nc.vector.tensor_mul(h2f_sb[:, :], r2_sb[:, :], mask_rep[:, :])
                nc.vector.reduce_sum(out=gv_sb[:, :], in_=h2f_sb[:, :],
                                     axis=mybir.AxisListType.X)

                # ---- head: dense(64, relu) -> dense(10) -> softmax ----
                d_ps = pool_out.tile([64, 1], F32, tag="outp")
                nc.tensor.matmul(out=d_ps[:, :], lhsT=wp_sb[0:32, DW],
                                 rhs=gv_sb[:, :], start=True, stop=True)
                d_sb = pool_misc.tile([64, 1], F32, tag="d")
                nc.scalar.activation(out=d_sb[:, :], in_=d_ps[:, :], func=AF.Relu,
                                     bias=wp_sb[0:64, DB], scale=1.0 / 128.0)
                lg_ps = pool_out.tile([1, 10], F32, tag="outp")
                nc.tensor.matmul(out=lg_ps[:, :], lhsT=d_sb[:, :],
                                 rhs=wp_sb[0:64, OW], start=True, stop=True)
                lg_sb = pool_misc.tile([1, 10], F32, tag="lg")
                nc.vector.tensor_add(lg_sb[:, :], lg_ps[:, :], wp_sb[0:1, OB])
                ex_sb = pool_misc.tile([1, 10], F32, tag="ex")
                ssum = pool_misc.tile([1, 1], F32, tag="ssum")
                nc.scalar.activation(out=ex_sb[:, :], in_=lg_sb[:, :], func=AF.Exp,
                                     accum_out=ssum[:, :])
                rs_sb = pool_misc.tile([1, 1], F32, tag="rs")
                nc.vector.reciprocal(rs_sb[:, :], ssum[:, :])
                probs = pool_misc.tile([1, 10], F32, tag="probs")
                nc.vector.tensor_scalar(
                    out=probs[:, :], in0=ex_sb[:, :], scalar1=rs_sb[0:1, 0:1],
                    scalar2=None, op0=ALU.mult)
                nc.sync.dma_start(out=out_d, in_=probs[:, :])

        if loop_n is not None and loop_n > 1:
            with tc.For_i(0, loop_n, 1):
                body()
        else:
            body()
    nc.compile()
    return nc


def prep_inputs(x, a, e, w1a, b1a, w1b, b1b, w1c, b1c, root1, bias1,
                w2a, b2a, w2b, b2b, w2c, b2c, root2, bias2, dw, db, ow, ob):
    """Host-side shard + layout prep. Returns in_maps (one per core)."""
    x = np.asarray(x, np.float32)
    a = np.asarray(a, np.float32)
    e = np.asarray(e, np.float32)
    # These biases are structurally zero in this problem (jnp.zeros in
    # setup_inputs); the device program relies on it (see module docstring).
    for b_ in (b1a, b1b, b1c, b2a, b2b, b2c):
        assert np.abs(np.asarray(b_)).max() == 0.0, "nonzero MLP bias unsupported"

    wpack = np.zeros((128, WCOLS), np.float32)
    for r in range(4):
        wpack[32 * r:32 * r + 8, W1A] = np.asarray(w1a)
        wpack[32 * r:32 * r + 8, W2A] = np.asarray(w2a)
    wpack[0:64, W1B] = np.asarray(w1b)
    wpack[64:128, W1B] = np.asarray(w1b)
    wpack[0:64, W2B] = np.asarray(w2b)
    wpack[64:128, W2B] = np.asarray(w2b)
    wpack[0:16, ROOT1] = np.asarray(root1)
    wpack[0:32, ROOT2] = np.asarray(root2)
    wpack[0:32, DW] = np.asarray(dw)
    wpack[0:64, OW] = np.asarray(ow)
    wpack[0:64, DB] = np.asarray(db).reshape(64, 1)
    wpack[0:1, BIAS1] = np.asarray(bias1).reshape(1, 32)
    wpack[0:1, BIAS2] = np.asarray(bias2).reshape(1, 32)
    wpack[0:1, OB] = np.asarray(ob).reshape(1, 10)

    w1p = np.ascontiguousarray(
        np.asarray(w1c).reshape(32, 16, 32).transpose(1, 0, 2).reshape(16, 1024))
    w2p = np.ascontiguousarray(
        np.asarray(w2c).reshape(32, 32, 32).transpose(1, 0, 2).reshape(32, 1024))

    in_maps = []
    for g in range(B):
        eA = np.ascontiguousarray(
            (e[g] * a[g][..., None]).transpose(2, 1, 0).reshape(S, E))
        xc = np.ascontiguousarray(x[g].T)  # [17, 128]; rows 0:16 feats, 16 mask
        in_maps.append(dict(eA=eA, xc=xc, wpack=wpack, w1p=w1p, w2p=w2p))
    return in_maps


_NC_CACHE = {}


def _get_nc(loop_n=None):
    key = loop_n
    if key not in _NC_CACHE:
        _NC_CACHE[key] = build_nc(loop_n)
    return _NC_CACHE[key]


def kernel(**inputs) -> np.ndarray:
    in_maps = prep_inputs(**inputs)
    nc = _get_nc()
    res = run_bass_kernel_spmd(nc, in_maps, core_ids=list(range(B)))
    out = np.concatenate([res.results[g]["out"] for g in range(B)], axis=0)
    return out.astype(np.float32)


# revision 10
# speedup vs baseline: 1.9286x; 1.9286x over previous
"""Trainium2 Bass kernel for nn_Net_64982855188859 (ECC graph-conv net).

Network (per graph g of B=8, N=128 nodes, F=16 feats, S=8 edge feats, C=32):
  mask = x[..., -1:], h = x[..., :16]
  h = leaky_relu(ECC1(h, a, e) * mask)          ECC: per-edge MLP -> filters
  h = relu(ECC2(h, a, e)) * mask
  out = softmax(relu(mean_nodes(h) @ dw) @ ow)

Key algebraic restructuring (exact, not approximate):
 1. The einsum out[i,c] = sum_j a[i,j] * sum_k h2[i,j,k] * xW[j,k,c] where
    xW[j,k,c] = sum_f x[j,f] * w1c[k, f*C+c]  (w1c factored through x, which
    turns the huge [N,N,F*C] per-edge filter tensor into a [N, 32*C] tensor).
 2. a[i,j] >= 0 is a per-edge scalar, so it commutes through the per-edge
    ReLU MLP rows:  a*relu(relu(e@w1a)@w1b) = relu(relu((a*e)@w1a)@w1b).
    The host pre-multiplies e by a (eA), removing all masking work on device.
    This requires the MLP biases to be zero, which they structurally are in
    this problem (asserted on host).
 3. Per-core work = one graph (data-parallel over B=8 across 8 cores).

Device layout (edge order j-major: edge = j*128 + i):
  eA_sb  [128, 4096]: partitions 32r+s (s<8) hold edge-feature s of row-tile r;
         free = (round g, 512 edges); 4-way row-tiled K=8 matmuls -> h1.
  h1     [128, 512] psum banks: (half, k') x (j4, i); relu -> sbuf.
  h2     quad-pair psum [128, 256]: partitions (j4, k), free (quad, i);
         relu -> h2A_sb [128, 4096] = 32 contraction chunks [(j4,k), i].
  xW/yW  [128 (j), 1024 (k,c)] via matmul, restructured to [(j4,k), (t,c)]
         chunks through a DRAM round-trip (pure-AP strided DMA).
  contraction: out1[c, i] += xWr[:, 32t:32t+32].T @ h2A[:, 128t:128t+128]
         accumulated over 32 chunks in one PSUM bank, + bias outer product
         + root term, all in one accumulation group.
"""

import numpy as np

import concourse.bass as bass
import concourse.bacc as bacc
import concourse.mybir as mybir
import concourse.tile as tile
from concourse.bass_utils import run_bass_kernel_spmd

F32 = mybir.dt.float32
BF16 = mybir.dt.bfloat16
AF = mybir.ActivationFunctionType
ALU = mybir.AluOpType

B, N, F, S, C = 8, 128, 16, 8, 32
E = N * N               # 16384 edges per graph
NROUND = 8              # h1 rounds; each processes 2048 edges (16 j's)

# wbf column layout ([128, 192] bf16, shared): MLP weight stacks
W1A = slice(0, 64)        # w1a stacked at rows 32r..32r+8
W1B = slice(64, 96)       # w1b at rows 0:64 and 64:128
W2A = slice(96, 160)
W2B = slice(160, 192)
WBF_COLS = 192
# wpack column layout ([128, WCOLS] fp32, shared): fp32 tail weights
ROOT1 = slice(0, 32)      # rows 0:16
ROOT2 = slice(32, 64)     # rows 0:32
DW = slice(64, 128)       # rows 0:32
OW = slice(128, 138)      # rows 0:64
DB = slice(138, 139)      # rows 0:64  (column vector)
BIAS1 = slice(144, 176)   # row 0 (row vector)
BIAS2 = slice(176, 208)   # row 0
OB = slice(208, 218)      # row 0
WCOLS = 224


def _ecc_mlp_layer(nc, tc, pools, wb_sb, eA_sb, wa_cols, wb_cols, h2A_sb, evict_ct):
    """Emit the per-edge MLP for one ECC layer: eA -> h1 -> h2A_sb chunks."""
    pool_h1p, pool_h1s, pool_qp = pools
    for g in range(NROUND):
        # --- h1: 4 row-tiled K=8 matmuls into 2 psum banks ---
        banks = [pool_h1p.tile([128, 512], F32, tag="h1p", name=f"h1p{g}a"),
                 pool_h1p.tile([128, 512], F32, tag="h1p", name=f"h1p{g}b")]
        for r in range(4):
            nc.tensor.matmul(
                out=banks[r // 2][64 * (r % 2):64 * (r % 2) + 64, :],
                lhsT=wb_sb[32 * r:32 * r + 8, wa_cols],
                rhs=eA_sb[32 * r:32 * r + 8, 512 * g:512 * (g + 1)],
                start=True, stop=True,
                tile_position=(32 * r, 64 * (r % 2)),
            )
        # --- relu evict to sbuf (alternate ACT/DVE) ---
        h1s = []
        for bi, bank in enumerate(banks):
            t = pool_h1s.tile([128, 512], BF16, tag="h1s", name=f"h1s{g}_{bi}")
            if (evict_ct[0] + bi) % 2 == 0:
                nc.scalar.activation(out=t[:, :], in_=bank[:, :], func=AF.Relu)
            else:
                nc.vector.tensor_scalar_max(t[:, :], bank[:, :], 0.0)
            h1s.append(t)
        evict_ct[0] += 1

        # --- h2: 16 matmuls K=64 into 2 quad-pair banks ---
        # qp[0] holds quads 4g+0 (from banks[0] half0) and 4g+2 (banks[1] half0)
        # qp[1] holds quads 4g+1 (banks[0] half1) and 4g+3 (banks[1] half1)
        qp = [pool_qp.tile([128, 256], F32, tag="qp", name=f"qp{g}a"),
              pool_qp.tile([128, 256], F32, tag="qp", name=f"qp{g}b")]
        for j4 in range(4):
            for r in range(4):
                half = r % 2            # rows 64*half .. +64
                bank_i = r // 2
                nc.tensor.matmul(
                    out=qp[half][32 * j4:32 * j4 + 32,
                                 128 * bank_i:128 * bank_i + 128],
                    lhsT=wb_sb[64 * half:64 * half + 64, wb_cols],
                    rhs=h1s[bank_i][64 * half:64 * half + 64,
                                    128 * j4:128 * j4 + 128],
                    start=True, stop=True,
                    tile_position=(64 * half, 32 * j4),
                )
        # --- relu evict quad pairs into h2A_sb chunks ---
        # qp[half] free 0:128 -> chunk t=4g+half, free 128:256 -> t=4g+2+half
        for half in range(2):
            base = h2A_sb[:, 512 * g + 128 * half:]
            out_ap = bass.AP(
                tensor=base.tensor, offset=base.offset,
                ap=[base.ap[0], [256, 2], [1, 128]],
            )
            if (evict_ct[0] + half) % 2 == 0:
                nc.scalar.activation(
                    out=out_ap, in_=qp[half][:, :].rearrange("p (q i) -> p q i", q=2),
                    func=AF.Relu)
            else:
                nc.vector.tensor_scalar_max(
                    out_ap, qp[half][:, :].rearrange("p (q i) -> p q i", q=2), 0.0)
        evict_ct[0] += 1


def _xw_restructure(nc, tc, pool_big, pool_xw, dram_pool, lhsT, rhs, name):
    """out [128 (j), 1024 (k,c)] = lhsT.T @ rhs, restructured to
    [(j4 k), (t c)] chunk layout via a DRAM round-trip. Returns sbuf tile."""
    ps = pool_big.tile([128, 1024], F32, tag="big")
    for h in range(2):
        nc.tensor.matmul(out=ps[:, 512 * h:512 * (h + 1)], lhsT=lhsT,
                         rhs=rhs[:, 512 * h:512 * (h + 1)], start=True, stop=True)
    flat = pool_xw.tile([128, 1024], BF16, tag=f"{name}_flat")
    nc.scalar.activation(out=flat[:, 0:512], in_=ps[:, 0:512], func=AF.Copy)
    nc.vector.tensor_copy(flat[:, 512:1024], ps[:, 512:1024])
    dscratch = dram_pool.tile([128, 1024], BF16, tag=f"{name}_dram")
    nc.sync.dma_start(out=dscratch[:, :], in_=flat[:, :])
    restr = pool_xw.tile([128, 1024], BF16, tag=f"{name}_restr")
    dsr = dscratch[:, :].rearrange("(t j4) (k c) -> j4 k t c", j4=4, c=32)
    for j4 in range(4):
        nc.sync.dma_start(
            out=restr[32 * j4:32 * j4 + 32, :].rearrange(
                "k (t c) -> k t c", c=32),
            in_=dsr[j4],
        )
    return restr


def _contraction(nc, out_psum, bias_row, ones_sb, xwr_sb, h2A_sb, root_lhsT, root_rhs):
    """out1[c, i] = bias ⊗ ones + sum_t xwr_t.T @ h2A_t + root.T @ root_rhs."""
    nc.tensor.matmul(out=out_psum[:, :], lhsT=bias_row, rhs=ones_sb,
                     start=True, stop=False, skip_group_check=True)
    for t in range(32):
        nc.tensor.matmul(
            out=out_psum[:, :],
            lhsT=xwr_sb[:, 32 * t:32 * t + 32],
            rhs=h2A_sb[:, 128 * t:128 * t + 128],
            start=False, stop=False, skip_group_check=True,
        )
    nc.tensor.matmul(out=out_psum[:, :], lhsT=root_lhsT, rhs=root_rhs,
                     start=False, stop=True, skip_group_check=True)


def build_nc(loop_n: int | None = None):
    nc = bacc.Bacc("TRN2", target_bir_lowering=False, debug=False)
    eA_d = nc.dram_tensor("eA", [S, E], BF16, kind="ExternalInput").ap()
    xc_d = nc.dram_tensor("xc", [17, 128], F32, kind="ExternalInput").ap()
    xcb_d = nc.dram_tensor("xcb", [16, 128], BF16, kind="ExternalInput").ap()
    wp_d = nc.dram_tensor("wpack", [128, WCOLS], F32, kind="ExternalInput").ap()
    wbf_d = nc.dram_tensor("wbf", [128, WBF_COLS], BF16, kind="ExternalInput").ap()
    w1p_d = nc.dram_tensor("w1p", [16, 1024], BF16, kind="ExternalInput").ap()
    w2p_d = nc.dram_tensor("w2p", [32, 1024], BF16, kind="ExternalInput").ap()
    out_d = nc.dram_tensor("out", [1, 10], F32, kind="ExternalOutput").ap()

    with tile.TileContext(nc) as tc:
        with (
            tc.tile_pool(name="consts", bufs=1) as consts,
            tc.tile_pool(name="eA", bufs=1) as pool_eA,
            tc.tile_pool(name="h1p", bufs=2, space="PSUM") as pool_h1p,
            tc.tile_pool(name="h1s", bufs=4) as pool_h1s,
            tc.tile_pool(name="qp", bufs=3, space="PSUM") as pool_qp,
            tc.tile_pool(name="big", bufs=1, space="PSUM") as pool_big,
            tc.tile_pool(name="outp", bufs=1, space="PSUM") as pool_out,
            tc.tile_pool(name="xw", bufs=1) as pool_xw,
            tc.tile_pool(name="h2A", bufs=1) as pool_h2A,
            tc.tile_pool(name="misc", bufs=1) as pool_misc,
            tc.tile_pool(name="dram", bufs=1, space="DRAM") as dram_pool,
        ):
            def body():
                # ---- load constants ----
                # MLP weights + eA first: the h1 matmuls only need these,
                # so PE can start while the remaining constants stream in.
                wb_sb = consts.tile([128, WBF_COLS], BF16)
                nc.sync.dma_start(out=wb_sb[:, :], in_=wbf_d)
                eA_sb = pool_eA.tile([128, 4096], BF16)
                eA_r = eA_d.rearrange("s (g r c) -> r s g c", g=8, r=4, c=512)
                for r in range(4):
                    nc.sync.dma_start(
                        out=eA_sb[32 * r:32 * r + 8, :].rearrange(
                            "s (g c) -> s g c", g=8),
                        in_=eA_r[r],
                    )
                w1p_sb = consts.tile([16, 1024], BF16)
                nc.sync.dma_start(out=w1p_sb[:, :], in_=w1p_d)
                xcb_sb = consts.tile([16, 128], BF16)
                nc.sync.dma_start(out=xcb_sb[:, :], in_=xcb_d)
                wp_sb = consts.tile([128, WCOLS], F32)
                nc.sync.dma_start(out=wp_sb[:, :], in_=wp_d)
                w2p_sb = consts.tile([32, 1024], BF16)
                nc.sync.dma_start(out=w2p_sb[:, :], in_=w2p_d)
                xc_sb = consts.tile([17, 128], F32)
                nc.sync.dma_start(out=xc_sb[:, :], in_=xc_d)
                xT = xc_sb[0:16, :]
                mask_rep = consts.tile([32, 128], F32)
                nc.sync.dma_start(
                    out=mask_rep[:, :],
                    in_=bass.AP(tensor=xc_d.tensor, offset=16 * 128,
                                ap=[[0, 32], [1, 128]]),
                )
                ones_sb = consts.tile([1, 128], F32)
                nc.vector.memset(ones_sb[:, :], 1.0)

                # ---- xW = x @ W1p, restructured ----
                xwr_sb = _xw_restructure(nc, tc, pool_big, pool_xw, dram_pool,
                                         xcb_sb[:, :], w1p_sb[:, :], "xw")

                # ---- layer-1 MLP over edges ----
                h2A_sb = pool_h2A.tile([128, 4096], BF16, tag="h2A_l1")
                evict_ct = [0]
                _ecc_mlp_layer(nc, tc, (pool_h1p, pool_h1s, pool_qp), wb_sb,
                               eA_sb, W1A, W1B, h2A_sb, evict_ct)

                # ---- layer-1 contraction + finish ----
                out1 = pool_out.tile([32, 128], F32, tag="outp")
                _contraction(nc, out1, wp_sb[0:1, BIAS1], ones_sb[:, :],
                             xwr_sb, h2A_sb, wp_sb[0:16, ROOT1], xT)
                z_sb = pool_misc.tile([32, 128], F32, tag="z")
                nc.vector.tensor_mul(z_sb[:, :], out1[:, :], mask_rep[:, :])
                y1_sb = pool_misc.tile([32, 128], F32, tag="y1")
                nc.vector.scalar_tensor_tensor(
                    out=y1_sb[:, :], in0=z_sb[:, :], scalar=0.05, in1=z_sb[:, :],
                    op0=ALU.mult, op1=ALU.max)

                # ---- yW = y1 @ W2p, restructured ----
                y1_bf = pool_misc.tile([32, 128], BF16, tag="y1bf")
                nc.vector.tensor_copy(y1_bf[:, :], y1_sb[:, :])
                ywr_sb = _xw_restructure(nc, tc, pool_big, pool_xw, dram_pool,
                                         y1_bf[:, :], w2p_sb[:, :], "yw")

                # ---- layer-2 MLP over edges ----
                g2A_sb = pool_h2A.tile([128, 4096], BF16, tag="h2A_l2")
                _ecc_mlp_layer(nc, tc, (pool_h1p, pool_h1s, pool_qp), wb_sb,
                               eA_sb, W2A, W2B, g2A_sb, evict_ct)

                # ---- layer-2 contraction + finish ----
                out2 = pool_out.tile([32, 128], F32, tag="outp")
                _contraction(nc, out2, wp_sb[0:1, BIAS2], ones_sb[:, :],
                             ywr_sb, g2A_sb, wp_sb[0:32, ROOT2], y1_sb[:, :])
                r2_sb = pool_misc.tile([32, 128], F32, tag="r2")
                nc.scalar.activation(out=r2_sb[:, :], in_=out2[:, :], func=AF.Relu)
                h2f_sb = pool_misc.tile([32, 128], F32, tag="h2f")
                gv_sb = pool_misc.tile([32, 1], F32, tag="gv")
                nc.vector.tensor_mul(h2f_sb[:, :], r2_sb[:, :], mask_rep[:, :])
                nc.vector.reduce_sum(out=gv_sb[:, :], in_=h2f_sb[:, :],
                                     axis=mybir.AxisListType.X)

                # ---- head: dense(64, relu) -> dense(10) -> softmax ----
                d_ps = pool_out.tile([64, 1], F32, tag="outp")
                nc.tensor.matmul(out=d_ps[:, :], lhsT=wp_sb[0:32, DW],
                                 rhs=gv_sb[:, :], start=True, stop=True)
                d_sb = pool_misc.tile([64, 1], F32, tag="d")
                nc.scalar.activation(out=d_sb[:, :], in_=d_ps[:, :], func=AF.Relu,
                                     bias=wp_sb[0:64, DB], scale=1.0 / 128.0)
                lg_ps = pool_out.tile([1, 10], F32, tag="outp")
                nc.tensor.matmul(out=lg_ps[:, :], lhsT=d_sb[:, :],
                                 rhs=wp_sb[0:64, OW], start=True, stop=True)
                lg_sb = pool_misc.tile([1, 10], F32, tag="lg")
                nc.vector.tensor_add(lg_sb[:, :], lg_ps[:, :], wp_sb[0:1, OB])
                ex_sb = pool_misc.tile([1, 10], F32, tag="ex")
                ssum = pool_misc.tile([1, 1], F32, tag="ssum")
                nc.scalar.activation(out=ex_sb[:, :], in_=lg_sb[:, :], func=AF.Exp,
                                     accum_out=ssum[:, :])
                rs_sb = pool_misc.tile([1, 1], F32, tag="rs")
                nc.vector.reciprocal(rs_sb[:, :], ssum[:, :])
                probs = pool_misc.tile([1, 10], F32, tag="probs")
                nc.vector.tensor_scalar(
                    out=probs[:, :], in0=ex_sb[:, :], scalar1=rs_sb[0:1, 0:1],
                    scalar2=None, op0=ALU.mult)
                nc.sync.dma_start(out=out_d, in_=probs[:, :])

            if loop_n is not None and loop_n > 1:
                with tc.For_i(0, loop_n, 1, hint_engines=(
                        mybir.EngineType.PE, mybir.EngineType.DVE,
                        mybir.EngineType.Activation, mybir.EngineType.SP)):
                    body()
            else:
                body()
    nc.compile()
    return nc


def prep_inputs(x, a, e, w1a, b1a, w1b, b1b, w1c, b1c, root1, bias1,
                w2a, b2a, w2b, b2b, w2c, b2c, root2, bias2, dw, db, ow, ob):
    """Host-side shard + layout prep. Returns in_maps (one per core)."""
    x = np.asarray(x, np.float32)
    a = np.asarray(a, np.float32)
    e = np.asarray(e, np.float32)
    # These biases are structurally zero in this problem (jnp.zeros in
    # setup_inputs); the device program relies on it (see module docstring).
    for b_ in (b1a, b1b, b1c, b2a, b2b, b2c):
        assert np.abs(np.asarray(b_)).max() == 0.0, "nonzero MLP bias unsupported"

    import ml_dtypes
    bf16 = ml_dtypes.bfloat16
    wbf = np.zeros((128, WBF_COLS), bf16)
    for r in range(4):
        wbf[32 * r:32 * r + 8, W1A] = np.asarray(w1a).astype(bf16)
        wbf[32 * r:32 * r + 8, W2A] = np.asarray(w2a).astype(bf16)
    wbf[0:64, W1B] = np.asarray(w1b).astype(bf16)
    wbf[64:128, W1B] = np.asarray(w1b).astype(bf16)
    wbf[0:64, W2B] = np.asarray(w2b).astype(bf16)
    wbf[64:128, W2B] = np.asarray(w2b).astype(bf16)
    wpack = np.zeros((128, WCOLS), np.float32)
    wpack[0:16, ROOT1] = np.asarray(root1)
    wpack[0:32, ROOT2] = np.asarray(root2)
    wpack[0:32, DW] = np.asarray(dw)
    wpack[0:64, OW] = np.asarray(ow)
    wpack[0:64, DB] = np.asarray(db).reshape(64, 1)
    wpack[0:1, BIAS1] = np.asarray(bias1).reshape(1, 32)
    wpack[0:1, BIAS2] = np.asarray(bias2).reshape(1, 32)
    wpack[0:1, OB] = np.asarray(ob).reshape(1, 10)

    w1p = np.ascontiguousarray(
        np.asarray(w1c).reshape(32, 16, 32).transpose(1, 0, 2).reshape(16, 1024)
    ).astype(bf16)
    w2p = np.ascontiguousarray(
        np.asarray(w2c).reshape(32, 32, 32).transpose(1, 0, 2).reshape(32, 1024)
    ).astype(bf16)

    in_maps = []
    for g in range(B):
        eA = np.ascontiguousarray(
            (e[g] * a[g][..., None]).transpose(2, 1, 0).reshape(S, E)).astype(bf16)
        xc = np.ascontiguousarray(x[g].T)  # [17, 128]; rows 0:16 feats, 16 mask
        xcb = np.ascontiguousarray(x[g].T[0:16]).astype(bf16)
        in_maps.append(dict(eA=eA, xc=xc, xcb=xcb, wpack=wpack, wbf=wbf,
                            w1p=w1p, w2p=w2p))
    return in_maps


_NC_CACHE = {}


def _get_nc(loop_n=None):
    key = loop_n
    if key not in _NC_CACHE:
        _NC_CACHE[key] = build_nc(loop_n)
    return _NC_CACHE[key]


def kernel(**inputs) -> np.ndarray:
    in_maps = prep_inputs(**inputs)
    nc = _get_nc()
    res = run_bass_kernel_spmd(nc, in_maps, core_ids=list(range(B)))
    out = np.concatenate([res.results[g]["out"] for g in range(B)], axis=0)
    return out.astype(np.float32)


# revision 12
# speedup vs baseline: 2.2430x; 1.1630x over previous
"""Trainium2 Bass kernel for nn_Net_64982855188859 (ECC graph-conv net).

Network (per graph g of B=8, N=128 nodes, F=16 feats, S=8 edge feats, C=32):
  mask = x[..., -1:], h = x[..., :16]
  h = leaky_relu(ECC1(h, a, e) * mask)          ECC: per-edge MLP -> filters
  h = relu(ECC2(h, a, e)) * mask
  out = softmax(relu(mean_nodes(h) @ dw) @ ow)

Key algebraic restructuring (exact, not approximate):
 1. The einsum out[i,c] = sum_j a[i,j] * sum_k h2[i,j,k] * xW[j,k,c] where
    xW[j,k,c] = sum_f x[j,f] * w1c[k, f*C+c]  (w1c factored through x, which
    turns the huge [N,N,F*C] per-edge filter tensor into a [N, 32*C] tensor).
 2. a[i,j] >= 0 is a per-edge scalar, so it commutes through the per-edge
    ReLU MLP rows:  a*relu(relu(e@w1a)@w1b) = relu(relu((a*e)@w1a)@w1b).
    The host pre-multiplies e by a (eA), removing all masking work on device.
    This requires the MLP biases to be zero, which they structurally are in
    this problem (asserted on host).
 3. Per-core work = one graph (data-parallel over B=8 across 8 cores).

Device layout (edge order j-major: edge = j*128 + i):
  eA_sb  [128, 4096]: partitions 32r+s (s<8) hold edge-feature s of row-tile r;
         free = (round g, 512 edges); 4-way row-tiled K=8 matmuls -> h1.
  h1     [128, 512] psum banks: (half, k') x (j4, i); relu -> sbuf.
  h2     quad-pair psum [128, 256]: partitions (j4, k), free (quad, i);
         relu -> h2A_sb [128, 4096] = 32 contraction chunks [(j4,k), i].
  xW/yW  [128 (j), 1024 (k,c)] via matmul, restructured to [(j4,k), (t,c)]
         chunks through a DRAM round-trip (pure-AP strided DMA).
  contraction: out1[c, i] += xWr[:, 32t:32t+32].T @ h2A[:, 128t:128t+128]
         accumulated over 32 chunks in one PSUM bank, + bias outer product
         + root term, all in one accumulation group.
"""

import numpy as np

import concourse.bass as bass
import concourse.bacc as bacc
import concourse.mybir as mybir
import concourse.tile as tile
from concourse.bass_utils import run_bass_kernel_spmd

F32 = mybir.dt.float32
BF16 = mybir.dt.bfloat16
AF = mybir.ActivationFunctionType
ALU = mybir.AluOpType

B, N, F, S, C = 8, 128, 16, 8, 32
E = N * N               # 16384 edges per graph
NROUND = 8              # h1 rounds; each processes 2048 edges (16 j's)

# wbf column layout ([128, 192] bf16, shared): MLP weight stacks
W1A = slice(0, 64)        # w1a stacked at rows 32r..32r+8
W1B = slice(64, 96)       # w1b at rows 0:64 and 64:128
W2A = slice(96, 160)
W2B = slice(160, 192)
WBF_COLS = 192
# wpack column layout ([128, WCOLS] fp32, shared): fp32 tail weights
ROOT1 = slice(0, 32)      # rows 0:16
ROOT2 = slice(32, 64)     # rows 0:32
DW = slice(64, 128)       # rows 0:32
OW = slice(128, 138)      # rows 0:64
DB = slice(138, 139)      # rows 0:64  (column vector)
BIAS1 = slice(144, 176)   # row 0 (row vector)
BIAS2 = slice(176, 208)   # row 0
OB = slice(208, 218)      # row 0
WCOLS = 224


def _ecc_mlp_layer(nc, tc, pools, wb_sb, eA_sb, wa_cols, wb_cols, h2A_sb, evict_ct):
    """Emit the per-edge MLP for one ECC layer: eA -> h1 -> h2A_sb chunks."""
    pool_h1p, pool_h1s, pool_qp = pools
    for g in range(NROUND):
        # --- h1: 4 row-tiled K=8 matmuls into 2 psum banks ---
        banks = [pool_h1p.tile([128, 512], F32, tag="h1p", name=f"h1p{g}a"),
                 pool_h1p.tile([128, 512], F32, tag="h1p", name=f"h1p{g}b")]
        for r in range(4):
            nc.tensor.matmul(
                out=banks[r // 2][64 * (r % 2):64 * (r % 2) + 64, :],
                lhsT=wb_sb[32 * r:32 * r + 8, wa_cols],
                rhs=eA_sb[32 * r:32 * r + 8, 512 * g:512 * (g + 1)],
                start=True, stop=True,
                tile_position=(32 * r, 64 * (r % 2)),
            )
        # --- relu evict to sbuf (alternate ACT/DVE) ---
        h1s = []
        for bi, bank in enumerate(banks):
            t = pool_h1s.tile([128, 512], BF16, tag="h1s", name=f"h1s{g}_{bi}")
            if (evict_ct[0] + bi) % 2 == 0:
                nc.scalar.activation(out=t[:, :], in_=bank[:, :], func=AF.Relu)
            else:
                nc.vector.tensor_scalar_max(t[:, :], bank[:, :], 0.0)
            h1s.append(t)
        evict_ct[0] += 1

        # --- h2: 16 matmuls K=64 into 2 quad-pair banks ---
        # qp[0] holds quads 4g+0 (from banks[0] half0) and 4g+2 (banks[1] half0)
        # qp[1] holds quads 4g+1 (banks[0] half1) and 4g+3 (banks[1] half1)
        qp = [pool_qp.tile([128, 256], F32, tag="qp", name=f"qp{g}a"),
              pool_qp.tile([128, 256], F32, tag="qp", name=f"qp{g}b")]
        for j4 in range(4):
            for r in range(4):
                half = r % 2            # rows 64*half .. +64
                bank_i = r // 2
                nc.tensor.matmul(
                    out=qp[half][32 * j4:32 * j4 + 32,
                                 128 * bank_i:128 * bank_i + 128],
                    lhsT=wb_sb[64 * half:64 * half + 64, wb_cols],
                    rhs=h1s[bank_i][64 * half:64 * half + 64,
                                    128 * j4:128 * j4 + 128],
                    start=True, stop=True,
                    tile_position=(64 * half, 32 * j4),
                )
        # --- relu evict quad pairs into h2A_sb chunks ---
        # qp[half] free 0:128 -> chunk t=4g+half, free 128:256 -> t=4g+2+half
        for half in range(2):
            base = h2A_sb[:, 512 * g + 128 * half:]
            out_ap = bass.AP(
                tensor=base.tensor, offset=base.offset,
                ap=[base.ap[0], [256, 2], [1, 128]],
            )
            if (evict_ct[0] + half) % 2 == 0:
                nc.scalar.activation(
                    out=out_ap, in_=qp[half][:, :].rearrange("p (q i) -> p q i", q=2),
                    func=AF.Relu)
            else:
                nc.vector.tensor_scalar_max(
                    out_ap, qp[half][:, :].rearrange("p (q i) -> p q i", q=2), 0.0)
        evict_ct[0] += 1


def _xw_restructure(nc, tc, pool_big, pool_xw, dram_pool, lhsT, rhs, name):
    """out [128 (j), 1024 (k,c)] = lhsT.T @ rhs, restructured to
    [(j4 k), (t c)] chunk layout via a DRAM round-trip. Returns sbuf tile."""
    ps = pool_big.tile([128, 1024], F32, tag="big")
    for h in range(2):
        nc.tensor.matmul(out=ps[:, 512 * h:512 * (h + 1)], lhsT=lhsT,
                         rhs=rhs[:, 512 * h:512 * (h + 1)], start=True, stop=True)
    flat = pool_xw.tile([128, 1024], BF16, tag=f"{name}_flat")
    nc.scalar.activation(out=flat[:, 0:512], in_=ps[:, 0:512], func=AF.Copy)
    nc.vector.tensor_copy(flat[:, 512:1024], ps[:, 512:1024])
    dscratch = dram_pool.tile([128, 1024], BF16, tag=f"{name}_dram")
    nc.sync.dma_start(out=dscratch[:, :], in_=flat[:, :])
    restr = pool_xw.tile([128, 1024], BF16, tag=f"{name}_restr")
    dsr = dscratch[:, :].rearrange("(t j4) (k c) -> j4 k t c", j4=4, c=32)
    for j4 in range(4):
        nc.sync.dma_start(
            out=restr[32 * j4:32 * j4 + 32, :].rearrange(
                "k (t c) -> k t c", c=32),
            in_=dsr[j4],
        )
    return restr


def _contraction(nc, out_psum, bias_row, ones_sb, xwr_sb, h2A_sb, root_lhsT, root_rhs):
    """out1[c, i] = bias ⊗ ones + sum_t xwr_t.T @ h2A_t + root.T @ root_rhs."""
    nc.tensor.matmul(out=out_psum[:, :], lhsT=bias_row, rhs=ones_sb,
                     start=True, stop=False, skip_group_check=True)
    for t in range(32):
        nc.tensor.matmul(
            out=out_psum[:, :],
            lhsT=xwr_sb[:, 32 * t:32 * t + 32],
            rhs=h2A_sb[:, 128 * t:128 * t + 128],
            start=False, stop=False, skip_group_check=True,
        )
    nc.tensor.matmul(out=out_psum[:, :], lhsT=root_lhsT, rhs=root_rhs,
                     start=False, stop=True, skip_group_check=True)


def build_nc(loop_n: int | None = None):
    nc = bacc.Bacc("TRN2", target_bir_lowering=False, debug=False)
    eA_d = nc.dram_tensor("eA", [S, E], BF16, kind="ExternalInput").ap()
    xc_d = nc.dram_tensor("xc", [17, 128], F32, kind="ExternalInput").ap()
    xcb_d = nc.dram_tensor("xcb", [16, 128], BF16, kind="ExternalInput").ap()
    wp_d = nc.dram_tensor("wpack", [128, WCOLS], F32, kind="ExternalInput").ap()
    wbf_d = nc.dram_tensor("wbf", [128, WBF_COLS], BF16, kind="ExternalInput").ap()
    w1p_d = nc.dram_tensor("w1p", [16, 1024], BF16, kind="ExternalInput").ap()
    w2p_d = nc.dram_tensor("w2p", [32, 1024], BF16, kind="ExternalInput").ap()
    out_d = nc.dram_tensor("out", [1, 10], F32, kind="ExternalOutput").ap()

    with tile.TileContext(nc) as tc:
        with (
            tc.tile_pool(name="consts", bufs=1) as consts,
            tc.tile_pool(name="eA", bufs=1) as pool_eA,
            tc.tile_pool(name="h1p", bufs=2, space="PSUM") as pool_h1p,
            tc.tile_pool(name="h1s", bufs=4) as pool_h1s,
            tc.tile_pool(name="qp", bufs=3, space="PSUM") as pool_qp,
            tc.tile_pool(name="big", bufs=1, space="PSUM") as pool_big,
            tc.tile_pool(name="outp", bufs=1, space="PSUM") as pool_out,
            tc.tile_pool(name="xw", bufs=1) as pool_xw,
            tc.tile_pool(name="h2A", bufs=1) as pool_h2A,
            tc.tile_pool(name="misc", bufs=1) as pool_misc,
            tc.tile_pool(name="dram", bufs=1, space="DRAM") as dram_pool,
        ):
            def body():
                # ---- load constants ----
                wp_sb = consts.tile([128, WCOLS], F32)
                nc.sync.dma_start(out=wp_sb[:, :], in_=wp_d)
                wb_sb = consts.tile([128, WBF_COLS], BF16)
                nc.sync.dma_start(out=wb_sb[:, :], in_=wbf_d)
                w1p_sb = consts.tile([16, 1024], BF16)
                nc.sync.dma_start(out=w1p_sb[:, :], in_=w1p_d)
                w2p_sb = consts.tile([32, 1024], BF16)
                nc.sync.dma_start(out=w2p_sb[:, :], in_=w2p_d)
                xc_sb = consts.tile([17, 128], F32)
                nc.sync.dma_start(out=xc_sb[:, :], in_=xc_d)
                xcb_sb = consts.tile([16, 128], BF16)
                nc.sync.dma_start(out=xcb_sb[:, :], in_=xcb_d)
                xT = xc_sb[0:16, :]
                mask_rep = consts.tile([32, 128], F32)
                nc.sync.dma_start(
                    out=mask_rep[:, :],
                    in_=bass.AP(tensor=xc_d.tensor, offset=16 * 128,
                                ap=[[0, 32], [1, 128]]),
                )
                ones_sb = consts.tile([1, 128], F32)
                nc.vector.memset(ones_sb[:, :], 1.0)

                # ---- load eA (row-stacked for 4-way row tiling) ----
                eA_sb = pool_eA.tile([128, 4096], BF16)
                eA_r = eA_d.rearrange("s (g r c) -> r s g c", g=8, r=4, c=512)
                for r in range(4):
                    nc.sync.dma_start(
                        out=eA_sb[32 * r:32 * r + 8, :].rearrange(
                            "s (g c) -> s g c", g=8),
                        in_=eA_r[r],
                    )

                # ---- xW = x @ W1p, restructured ----
                xwr_sb = _xw_restructure(nc, tc, pool_big, pool_xw, dram_pool,
                                         xcb_sb[:, :], w1p_sb[:, :], "xw")

                # ---- layer-1 MLP over edges ----
                h2A_sb = pool_h2A.tile([128, 4096], BF16, tag="h2A_l1")
                evict_ct = [0]
                _ecc_mlp_layer(nc, tc, (pool_h1p, pool_h1s, pool_qp), wb_sb,
                               eA_sb, W1A, W1B, h2A_sb, evict_ct)

                # ---- layer-1 contraction + finish ----
                out1 = pool_out.tile([32, 128], F32, tag="outp")
                _contraction(nc, out1, wp_sb[0:1, BIAS1], ones_sb[:, :],
                             xwr_sb, h2A_sb, wp_sb[0:16, ROOT1], xT)
                z_sb = pool_misc.tile([32, 128], F32, tag="z")
                nc.vector.tensor_mul(z_sb[:, :], out1[:, :], mask_rep[:, :])
                y1_sb = pool_misc.tile([32, 128], F32, tag="y1")
                nc.vector.scalar_tensor_tensor(
                    out=y1_sb[:, :], in0=z_sb[:, :], scalar=0.05, in1=z_sb[:, :],
                    op0=ALU.mult, op1=ALU.max)

                # ---- yW = y1 @ W2p, restructured ----
                y1_bf = pool_misc.tile([32, 128], BF16, tag="y1bf")
                nc.vector.tensor_copy(y1_bf[:, :], y1_sb[:, :])
                ywr_sb = _xw_restructure(nc, tc, pool_big, pool_xw, dram_pool,
                                         y1_bf[:, :], w2p_sb[:, :], "yw")

                # ---- layer-2 MLP over edges ----
                g2A_sb = pool_h2A.tile([128, 4096], BF16, tag="h2A_l2")
                _ecc_mlp_layer(nc, tc, (pool_h1p, pool_h1s, pool_qp), wb_sb,
                               eA_sb, W2A, W2B, g2A_sb, evict_ct)

                # ---- layer-2 contraction + finish ----
                out2 = pool_out.tile([32, 128], F32, tag="outp")
                _contraction(nc, out2, wp_sb[0:1, BIAS2], ones_sb[:, :],
                             ywr_sb, g2A_sb, wp_sb[0:32, ROOT2], y1_sb[:, :])
                r2_sb = pool_misc.tile([32, 128], F32, tag="r2")
                nc.scalar.activation(out=r2_sb[:, :], in_=out2[:, :], func=AF.Relu)
                h2f_sb = pool_misc.tile([32, 128], F32, tag="h2f")
                gv_sb = pool_misc.tile([32, 1], F32, tag="gv")
                nc.vector.tensor_mul(h2f_sb[:, :], r2_sb[:, :], mask_rep[:, :])
                nc.vector.reduce_sum(out=gv_sb[:, :], in_=h2f_sb[:, :],
                                     axis=mybir.AxisListType.X)

                # ---- head: dense(64, relu) -> dense(10) -> softmax ----
                d_ps = pool_out.tile([64, 1], F32, tag="outp")
                nc.tensor.matmul(out=d_ps[:, :], lhsT=wp_sb[0:32, DW],
                                 rhs=gv_sb[:, :], start=True, stop=True)
                d_sb = pool_misc.tile([64, 1], F32, tag="d")
                nc.scalar.activation(out=d_sb[:, :], in_=d_ps[:, :], func=AF.Relu,
                                     bias=wp_sb[0:64, DB], scale=1.0 / 128.0)
                lg_ps = pool_out.tile([1, 10], F32, tag="outp")
                nc.tensor.matmul(out=lg_ps[:, :], lhsT=d_sb[:, :],
                                 rhs=wp_sb[0:64, OW], start=True, stop=True)
                lg_sb = pool_misc.tile([1, 10], F32, tag="lg")
                nc.vector.tensor_add(lg_sb[:, :], lg_ps[:, :], wp_sb[0:1, OB])
                ex_sb = pool_misc.tile([1, 10], F32, tag="ex")
                ssum = pool_misc.tile([1, 1], F32, tag="ssum")
                nc.scalar.activation(out=ex_sb[:, :], in_=lg_sb[:, :], func=AF.Exp,
                                     accum_out=ssum[:, :])
                rs_sb = pool_misc.tile([1, 1], F32, tag="rs")
                nc.vector.reciprocal(rs_sb[:, :], ssum[:, :])
                probs = pool_misc.tile([1, 10], F32, tag="probs")
                nc.vector.tensor_scalar(
                    out=probs[:, :], in0=ex_sb[:, :], scalar1=rs_sb[0:1, 0:1],
                    scalar2=None, op0=ALU.mult)
                nc.sync.dma_start(out=out_d, in_=probs[:, :])

            if loop_n is not None and loop_n > 1:
                with tc.For_i(0, loop_n, 1, hint_engines=(
                        mybir.EngineType.PE, mybir.EngineType.DVE,
                        mybir.EngineType.Activation, mybir.EngineType.SP)):
                    body()
            else:
                body()
    nc.compile()
    return nc


def prep_inputs(x, a, e, w1a, b1a, w1b, b1b, w1c, b1c, root1, bias1,
                w2a, b2a, w2b, b2b, w2c, b2c, root2, bias2, dw, db, ow, ob):
    """Host-side shard + layout prep. Returns in_maps (one per core)."""
    x = np.asarray(x, np.float32)
    a = np.asarray(a, np.float32)
    e = np.asarray(e, np.float32)
    # These biases are structurally zero in this problem (jnp.zeros in
    # setup_inputs); the device program relies on it (see module docstring).
    for b_ in (b1a, b1b, b1c, b2a, b2b, b2c):
        assert np.abs(np.asarray(b_)).max() == 0.0, "nonzero MLP bias unsupported"

    import ml_dtypes
    bf16 = ml_dtypes.bfloat16
    wbf = np.zeros((128, WBF_COLS), bf16)
    for r in range(4):
        wbf[32 * r:32 * r + 8, W1A] = np.asarray(w1a).astype(bf16)
        wbf[32 * r:32 * r + 8, W2A] = np.asarray(w2a).astype(bf16)
    wbf[0:64, W1B] = np.asarray(w1b).astype(bf16)
    wbf[64:128, W1B] = np.asarray(w1b).astype(bf16)
    wbf[0:64, W2B] = np.asarray(w2b).astype(bf16)
    wbf[64:128, W2B] = np.asarray(w2b).astype(bf16)
    wpack = np.zeros((128, WCOLS), np.float32)
    wpack[0:16, ROOT1] = np.asarray(root1)
    wpack[0:32, ROOT2] = np.asarray(root2)
    wpack[0:32, DW] = np.asarray(dw)
    wpack[0:64, OW] = np.asarray(ow)
    wpack[0:64, DB] = np.asarray(db).reshape(64, 1)
    wpack[0:1, BIAS1] = np.asarray(bias1).reshape(1, 32)
    wpack[0:1, BIAS2] = np.asarray(bias2).reshape(1, 32)
    wpack[0:1, OB] = np.asarray(ob).reshape(1, 10)

    w1p = np.ascontiguousarray(
        np.asarray(w1c).reshape(32, 16, 32).transpose(1, 0, 2).reshape(16, 1024)
    ).astype(bf16)
    w2p = np.ascontiguousarray(
        np.asarray(w2c).reshape(32, 32, 32).transpose(1, 0, 2).reshape(32, 1024)
    ).astype(bf16)

    in_maps = []
    for g in range(B):
        eA = np.ascontiguousarray(
            (e[g] * a[g][..., None]).transpose(2, 1, 0).reshape(S, E)).astype(bf16)
        xc = np.ascontiguousarray(x[g].T)  # [17, 128]; rows 0:16 feats, 16 mask
        xcb = np.ascontiguousarray(x[g].T[0:16]).astype(bf16)
        in_maps.append(dict(eA=eA, xc=xc, xcb=xcb, wpack=wpack, wbf=wbf,
                            w1p=w1p, w2p=w2p))
    return in_maps


_NC_CACHE = {}


def _get_nc(loop_n=None):
    key = loop_n
    if key not in _NC_CACHE:
        _NC_CACHE[key] = build_nc(loop_n)
    return _NC_CACHE[key]


def kernel(**inputs) -> np.ndarray:
    in_maps = prep_inputs(**inputs)
    nc = _get_nc()
    # The axon-tunneled device occasionally reports a transient
    # "exec unit unrecoverable" on the first dispatch after idle; a retry on
    # a fresh dispatch has always succeeded, so try up to 3 times.
    last = None
    for _ in range(3):
        try:
            res = run_bass_kernel_spmd(nc, in_maps, core_ids=list(range(B)))
            out = np.concatenate(
                [res.results[g]["out"] for g in range(B)], axis=0)
            return out.astype(np.float32)
        except Exception as ex:  # noqa: BLE001
            last = ex
    raise last
